# revision 1
# baseline (speedup 1.0000x reference)
"""CenterLoss Trainium2 kernel.

Reference computation (see problem statement):
    feats  [N=4096, D=96]  = features.reshape(-1, 96)          (float64 in ref)
    label  [N]             = argmax(predicts, axis=-1)          (fp32 argmax)
    dist_n                 = ||feats_n||^2 + ||c_{l_n}||^2 - 2 feats_n . c_{l_n}
                           = ||feats_n - c_{l_n}||^2
    loss = (sum_n clip(dist_n, 1e-12, 1e12) + (N*C - N) * 1e-12) / N
         -- the (C-1)*1e-12 term comes from clip() lifting the masked-out
            zeros of the [N, C] matrix to 1e-12 each.

Only the labeled column of the [N, C] distance matrix survives the mask, so
the kernel never materializes it: per 128-sample tile it
  1. streams predicts [128, 6625] into SBUF (the dominant cost, ~13.6MB/core),
  2. argmax along the free axis with DVE max / max_index,
  3. indirect-DMA gathers centers[label] rows,
  4. squares (features - gathered) on ACT with accum_out giving the
     per-sample squared distance,
  5. clamps, reduces across partitions with a ones-vector matmul.
Each of the 8 cores handles 512 samples; the host sums the 8 partial sums in
float64 and adds the (C-1)*1e-12 clip constant.
"""

import numpy as np

import concourse.bass as bass
import concourse.mybir as mybir
from concourse import bacc
from concourse.bass_utils import run_bass_kernel_spmd
from concourse.tile import TileContext

NUM_CLASSES = 6625
FEAT_DIM = 96
N_CORES = 8
N_TOTAL = 64 * 64          # 4096 samples
NS = N_TOTAL // N_CORES    # 512 samples per core
P = 128                    # partitions
NTILES = NS // P           # 4 tiles of 128 samples per core
CLAMP_MIN = 1e-12
CLAMP_MAX = 1e12

_NC_CACHE = {}


def _build_nc(
    reps=1,
    pred_bufs=4,
    pass1="hierg",
    dma_only=False,
    fake_gather=False,
    seg_w=128,
    # batching the 4 feature loads into one strided DMA measured ~13 us/rep
    # WORSE (3-level AP descriptor structure); keep per-tile contiguous loads
    batch_feat=False,
):
    # seg_w: argmax group width. 128 measured best on HW: narrower groups cut
    # the value-search width but the 3D reduce pays a per-group pipeline
    # restart (208 groups of 32 was ~15us/rep slower than 52 groups of 128).
    # reps>1 repeats the whole per-core computation; used only by the
    # benchmark harness to measure steady-state per-iteration device time
    # as the slope between rep counts (cancels launch + kernel-tail cost).
    # pass1: engine strategy for the argmax -
    #   "dve"  - InstMax + full-width InstMaxIndex (two 1x passes)
    #   "hier" - hierarchical: one reduce_max pass over [128,53,125] ->
    #            group maxes, tiny max_index picks the winning group, an
    #            indirect DMA gathers each row's 125-wide segment, and a
    #            tiny max_index finds the in-group position. DVE cost drops
    #            from two full passes to one.
    # dma_only: benchmark variant that loads predicts but skips the argmax,
    #   to measure the pure DMA floor.
    nc = bacc.Bacc("TRN2", target_bir_lowering=False)
    feats = nc.dram_tensor(
        "features", [NS, FEAT_DIM], mybir.dt.float32, kind="ExternalInput"
    )
    preds = nc.dram_tensor(
        "predicts", [NS, NUM_CLASSES], mybir.dt.float32, kind="ExternalInput"
    )
    cents = nc.dram_tensor(
        "centers", [NUM_CLASSES, FEAT_DIM], mybir.dt.float32, kind="ExternalInput"
    )
    out = nc.dram_tensor("out", [1, 1], mybir.dt.float32, kind="ExternalOutput")

    with TileContext(nc) as tc:
        with (
            tc.tile_pool(name="pred", bufs=pred_bufs) as pred_pool,
            tc.tile_pool(name="small", bufs=3) as small_pool,
            tc.tile_pool(name="persist", bufs=1) as persist_pool,
            tc.tile_pool(name="psum", bufs=1, space="PSUM") as psum_pool,
        ):
            ones = persist_pool.tile([P, 1], mybir.dt.float32)
            nc.vector.memset(ones[:], 1.0)

            G, SEG = 53, 125  # 53 * 125 == 6625
            preds_flat = preds[:].rearrange("n (g k) -> (n g) k", k=SEG)
            if pass1 in ("hier", "hierb"):
                # rowbase[p, j] = (j*128 + p) * G, as fp32 for ACT bias use
                rowbase_i = persist_pool.tile([P, NTILES], mybir.dt.int32)
                nc.gpsimd.iota(
                    rowbase_i[:],
                    pattern=[[P * G, NTILES]],
                    base=0,
                    channel_multiplier=G,
                )
                rowbase_f = persist_pool.tile([P, NTILES], mybir.dt.float32)
                nc.vector.tensor_copy(rowbase_f[:], rowbase_i[:])

            if pass1 == "hierb" and not dma_only:
                for _rep in range(reps):
                    dacc = persist_pool.tile([P, NTILES], mybir.dt.float32, tag="dacc")
                    offs_all = small_pool.tile([P, NTILES], mybir.dt.int32, tag="offs")
                    rmax8s = []
                    g8fs = []
                    # phase A: per-tile group-max + winning group
                    for j in range(NTILES):
                        rows = slice(j * P, (j + 1) * P)
                        ptile = pred_pool.tile([P, NUM_CLASSES], mybir.dt.float32)
                        nc.sync.dma_start(out=ptile[:], in_=preds[rows, :])
                        gmax = small_pool.tile([P, G], mybir.dt.float32)
                        nc.vector.reduce_max(
                            gmax[:],
                            ptile[:].rearrange("p (g k) -> p g k", k=SEG),
                            axis=mybir.AxisListType.X,
                        )
                        rmax = small_pool.tile([P, 1], mybir.dt.float32)
                        nc.vector.reduce_max(
                            rmax[:], gmax[:], axis=mybir.AxisListType.X
                        )
                        rmax8 = small_pool.tile([P, 8], mybir.dt.float32, tag=f"rmax8_{j}")
                        nc.scalar.activation(
                            rmax8[:],
                            rmax[:].to_broadcast([P, 8]),
                            mybir.ActivationFunctionType.Copy,
                        )
                        g8 = small_pool.tile([P, 8], mybir.dt.uint32)
                        nc.vector.max_index(g8[:], rmax8[:], gmax[:])
                        g8f = small_pool.tile([P, 1], mybir.dt.float32, tag=f"g8f_{j}")
                        nc.vector.tensor_copy(g8f[:], g8[:, 0:1])
                        offsf = small_pool.tile([P, 1], mybir.dt.float32)
                        nc.scalar.activation(
                            offsf[:],
                            g8f[:],
                            mybir.ActivationFunctionType.Identity,
                            bias=rowbase_f[:, j : j + 1],
                        )
                        nc.vector.tensor_copy(offs_all[:, j : j + 1], offsf[:])
                        rmax8s.append(rmax8)
                        g8fs.append(g8f)
                    # phase B: one batched segment gather for all 4 tiles
                    seg_all = small_pool.tile([P, NTILES, SEG], mybir.dt.float32)
                    nc.gpsimd.indirect_dma_start(
                        out=seg_all[:],
                        out_offset=None,
                        in_=preds_flat,
                        in_offset=bass.IndirectOffsetOnAxis(
                            ap=offs_all[:, 0:NTILES], axis=0
                        ),
                    )
                    idx_all = small_pool.tile([P, NTILES], mybir.dt.uint32, tag="idxall")
                    for j in range(NTILES):
                        k8 = small_pool.tile([P, 8], mybir.dt.uint32)
                        nc.vector.max_index(k8[:], rmax8s[j][:], seg_all[:, j, :])
                        k8f = small_pool.tile([P, 1], mybir.dt.float32)
                        nc.vector.tensor_copy(k8f[:], k8[:, 0:1])
                        idxf = small_pool.tile([P, 1], mybir.dt.float32)
                        nc.scalar.activation(
                            idxf[:],
                            g8fs[j][:],
                            mybir.ActivationFunctionType.Identity,
                            scale=float(SEG),
                            bias=k8f[:],
                        )
                        nc.vector.tensor_copy(idx_all[:, j : j + 1], idxf[:])
                    # phase C: one batched centers gather + distances
                    call = small_pool.tile([P, NTILES, FEAT_DIM], mybir.dt.float32)
                    nc.gpsimd.indirect_dma_start(
                        out=call[:],
                        out_offset=None,
                        in_=cents[:],
                        in_offset=bass.IndirectOffsetOnAxis(
                            ap=idx_all[:, 0:NTILES], axis=0
                        ),
                    )
                    fall = small_pool.tile([P, NTILES, FEAT_DIM], mybir.dt.float32)
                    nc.sync.dma_start(
                        out=fall[:],
                        in_=feats[:].rearrange("(j p) d -> p j d", p=P),
                    )
                    dall = small_pool.tile([P, NTILES, FEAT_DIM], mybir.dt.float32)
                    nc.vector.tensor_tensor(
                        out=dall[:],
                        in0=fall[:],
                        in1=call[:],
                        op=mybir.AluOpType.subtract,
                    )
                    for j in range(NTILES):
                        sq = small_pool.tile([P, FEAT_DIM], mybir.dt.float32)
                        nc.scalar.activation(
                            sq[:],
                            dall[:, j, :],
                            mybir.ActivationFunctionType.Square,
                            accum_out=dacc[:, j : j + 1],
                        )
                    _final_reduce(nc, persist_pool, psum_pool, dacc, ones, out)

            for _rep in range(reps if pass1 != "hierb" or dma_only else 0):
                dacc = persist_pool.tile([P, NTILES], mybir.dt.float32, tag="dacc")
                for j in range(NTILES):
                    rows = slice(j * P, (j + 1) * P)

                    if dma_only:
                        # aligned-pitch tile, same as the hierg data path
                        ptile = pred_pool.tile([P, 6656], mybir.dt.float32)
                        nc.sync.dma_start(
                            out=ptile[:, 0:NUM_CLASSES], in_=preds[rows, :]
                        )
                        nc.vector.reduce_max(
                            dacc[:, j : j + 1], ptile[:, 0:8],
                            axis=mybir.AxisListType.X,
                        )
                        continue

                    if pass1 == "hierg" and not dma_only:
                        # padded to (6656/seg_w) groups x seg_w cols; pad
                        # filled on ACT to keep the vector engine free
                        ptile = pred_pool.tile([P, 6656], mybir.dt.float32)
                        nc.sync.dma_start(
                            out=ptile[:, 0:NUM_CLASSES], in_=preds[rows, :]
                        )
                        nc.scalar.activation(
                            ptile[:, NUM_CLASSES:],
                            ones[:, 0:1].to_broadcast([P, 6656 - NUM_CLASSES]),
                            mybir.ActivationFunctionType.Copy,
                            scale=-1e30,
                        )
                    else:
                        ptile = pred_pool.tile([P, NUM_CLASSES], mybir.dt.float32)
                        nc.sync.dma_start(out=ptile[:], in_=preds[rows, :])

                    if dma_only:
                        # touch a sliver so the load isn't dead
                        nc.vector.reduce_max(
                            dacc[:, j : j + 1], ptile[:, 0:8],
                            axis=mybir.AxisListType.X,
                        )
                        continue

                    if pass1 == "hierg":
                        # groups of seg_w columns (padded with -1e30 so class
                        # index == g*seg_w + k, recoverable with bit ops)
                        SP = seg_w
                        GP = 6656 // SP
                        SHIFT = SP.bit_length() - 1
                        assert 1 << SHIFT == SP
                        gmax = small_pool.tile([P, GP], mybir.dt.float32)
                        nc.vector.reduce_max(
                            gmax[:],
                            ptile[:].rearrange("p (g k) -> p g k", k=SP),
                            axis=mybir.AxisListType.X,
                        )
                        # top-8 group maxes, sorted desc: slot 0 is the row
                        # max, and the tile doubles as max_index's in_max
                        rmax8 = small_pool.tile([P, 8], mybir.dt.float32)
                        nc.vector.max(rmax8[:], gmax[:])
                        g8 = small_pool.tile([P, 8], mybir.dt.uint16)
                        nc.vector.max_index(g8[:], rmax8[:], gmax[:])
                        # gather each partition's winning 128-wide group from
                        # SBUF on GpSimd. ap_gather broadcasts each of the 16
                        # partition indices to the whole 16-partition group, so
                        # partition p's own winning group lands at block p%16;
                        # the row max value only occurs in p's own group (any
                        # duplicate block is that same group, with the value at
                        # the same in-block position), so a value search over
                        # all 16 blocks yields the right in-group position.
                        blocks = small_pool.tile([P, 16, SP], mybir.dt.float32)
                        nc.gpsimd.ap_gather(
                            out_ap=blocks[:],
                            in_ap=ptile[:].rearrange("p (g k) -> p g k", k=SP),
                            idxs_ap=g8[:, 0:1].bitcast(mybir.dt.int16),
                            channels=P,
                            num_elems=GP,
                            d=SP,
                            num_idxs=16,
                        )
                        f8 = small_pool.tile([P, 8], mybir.dt.uint16)
                        nc.vector.max_index(
                            f8[:],
                            rmax8[:],
                            blocks[:].rearrange("p a b -> p (a b)"),
                        )
                        # class index = (g << SHIFT) + (found & (SP-1))
                        kmod = small_pool.tile([P, 1], mybir.dt.uint16)
                        nc.vector.tensor_scalar(
                            out=kmod[:],
                            in0=f8[:, 0:1],
                            scalar1=SP - 1,
                            scalar2=None,
                            op0=mybir.AluOpType.bitwise_and,
                        )
                        gshl = small_pool.tile([P, 1], mybir.dt.uint16)
                        nc.vector.tensor_scalar(
                            out=gshl[:],
                            in0=g8[:, 0:1],
                            scalar1=SHIFT,
                            scalar2=None,
                            op0=mybir.AluOpType.logical_shift_left,
                        )
                        idx8 = small_pool.tile([P, 1], mybir.dt.uint32)
                        nc.vector.tensor_tensor(
                            out=idx8[:],
                            in0=gshl[:],
                            in1=kmod[:],
                            op=mybir.AluOpType.add,
                        )
                    elif pass1 == "hier":
                        gmax = small_pool.tile([P, G], mybir.dt.float32)
                        nc.vector.reduce_max(
                            gmax[:],
                            ptile[:].rearrange("p (g k) -> p g k", k=SEG),
                            axis=mybir.AxisListType.X,
                        )
                        rmax = small_pool.tile([P, 1], mybir.dt.float32)
                        nc.vector.reduce_max(
                            rmax[:], gmax[:], axis=mybir.AxisListType.X
                        )
                        rmax8 = small_pool.tile([P, 8], mybir.dt.float32)
                        nc.scalar.activation(
                            rmax8[:],
                            rmax[:].to_broadcast([P, 8]),
                            mybir.ActivationFunctionType.Copy,
                        )
                        g8 = small_pool.tile([P, 8], mybir.dt.uint32)
                        nc.vector.max_index(g8[:], rmax8[:], gmax[:])
                        g8f = small_pool.tile([P, 1], mybir.dt.float32)
                        nc.vector.tensor_copy(g8f[:], g8[:, 0:1])
                        # offset into preds_flat: row*G + g
                        offsf = small_pool.tile([P, 1], mybir.dt.float32)
                        nc.scalar.activation(
                            offsf[:],
                            g8f[:],
                            mybir.ActivationFunctionType.Identity,
                            bias=rowbase_f[:, j : j + 1],
                        )
                        offsi = small_pool.tile([P, 1], mybir.dt.int32)
                        nc.vector.tensor_copy(offsi[:], offsf[:])
                        seg = small_pool.tile([P, SEG], mybir.dt.float32)
                        if fake_gather:
                            # benchmark probe: fixed-window read instead of a
                            # per-row indirect gather (wrong results)
                            nc.vector.tensor_copy(seg[:], ptile[:, 0:SEG])
                        else:
                            nc.gpsimd.indirect_dma_start(
                                out=seg[:],
                                out_offset=None,
                                in_=preds_flat,
                                in_offset=bass.IndirectOffsetOnAxis(
                                    ap=offsi[:, 0:1], axis=0
                                ),
                            )
                        k8 = small_pool.tile([P, 8], mybir.dt.uint32)
                        nc.vector.max_index(k8[:], rmax8[:], seg[:])
                        k8f = small_pool.tile([P, 1], mybir.dt.float32)
                        nc.vector.tensor_copy(k8f[:], k8[:, 0:1])
                        # class index: g*SEG + k
                        idxf = small_pool.tile([P, 1], mybir.dt.float32)
                        nc.scalar.activation(
                            idxf[:],
                            g8f[:],
                            mybir.ActivationFunctionType.Identity,
                            scale=float(SEG),
                            bias=k8f[:],
                        )
                        idx8 = small_pool.tile([P, 1], mybir.dt.uint32)
                        nc.vector.tensor_copy(idx8[:], idxf[:])
                    else:
                        max8 = small_pool.tile([P, 8], mybir.dt.float32)
                        idx8 = small_pool.tile([P, 8], mybir.dt.uint32)
                        nc.vector.max(max8[:], ptile[:])
                        nc.vector.max_index(idx8[:], max8[:], ptile[:])

                    if pass1 == "hierg" and batch_feat:
                        # all 512 feature rows in one strided DMA per rep
                        if j == 0:
                            fall = small_pool.tile(
                                [P, NTILES, FEAT_DIM], mybir.dt.float32, tag="fall"
                            )
                            nc.sync.dma_start(
                                out=fall[:],
                                in_=feats[:].rearrange("(j p) d -> p j d", p=P),
                            )
                        ftile = fall[:, j, :]
                    else:
                        ftile_t = small_pool.tile([P, FEAT_DIM], mybir.dt.float32)
                        nc.sync.dma_start(out=ftile_t[:], in_=feats[rows, :])
                        ftile = ftile_t[:]

                    ctile = small_pool.tile([P, FEAT_DIM], mybir.dt.float32)
                    if fake_gather:
                        nc.sync.dma_start(out=ctile[:], in_=cents[0:P, :])
                    else:
                        nc.gpsimd.indirect_dma_start(
                            out=ctile[:],
                            out_offset=None,
                            in_=cents[:],
                            in_offset=bass.IndirectOffsetOnAxis(
                                ap=idx8[:, 0:1], axis=0
                            ),
                        )

                    diff = small_pool.tile([P, FEAT_DIM], mybir.dt.float32)
                    nc.vector.tensor_tensor(
                        out=diff[:],
                        in0=ftile,
                        in1=ctile[:],
                        op=mybir.AluOpType.subtract,
                    )
                    sq = small_pool.tile([P, FEAT_DIM], mybir.dt.float32)
                    nc.scalar.activation(
                        sq[:],
                        diff[:],
                        mybir.ActivationFunctionType.Square,
                        accum_out=dacc[:, j : j + 1],
                    )

                _final_reduce(nc, persist_pool, psum_pool, dacc, ones, out)

    nc.compile()
    return nc


def _final_reduce(nc, persist_pool, psum_pool, dacc, ones, out):
    dclamp = persist_pool.tile([P, NTILES], mybir.dt.float32, tag="dclamp")
    nc.vector.tensor_scalar(
        out=dclamp[:],
        in0=dacc[:],
        scalar1=CLAMP_MIN,
        scalar2=CLAMP_MAX,
        op0=mybir.AluOpType.max,
        op1=mybir.AluOpType.min,
    )
    dsum = persist_pool.tile([P, 1], mybir.dt.float32, tag="dsum")
    nc.vector.reduce_sum(dsum[:], dclamp[:], axis=mybir.AxisListType.X)

    res_psum = psum_pool.tile([1, 1], mybir.dt.float32)
    nc.tensor.matmul(res_psum[:], lhsT=dsum[:], rhs=ones[:], start=True, stop=True)
    res_sb = persist_pool.tile([1, 1], mybir.dt.float32, tag="res_sb")
    nc.vector.tensor_copy(res_sb[:], res_psum[:])
    nc.sync.dma_start(out=out[:], in_=res_sb[:])


def _get_nc():
    if "nc" not in _NC_CACHE:
        _NC_CACHE["nc"] = _build_nc()
    return _NC_CACHE["nc"]


def kernel(features, predicts, centers):
    features = np.ascontiguousarray(np.asarray(features, dtype=np.float32))
    predicts = np.ascontiguousarray(np.asarray(predicts, dtype=np.float32))
    centers = np.ascontiguousarray(np.asarray(centers, dtype=np.float32))

    feats = features.reshape(N_TOTAL, FEAT_DIM)
    preds = predicts.reshape(N_TOTAL, NUM_CLASSES)

    in_maps = []
    for c in range(N_CORES):
        rows = slice(c * NS, (c + 1) * NS)
        in_maps.append(
            {
                "features": np.ascontiguousarray(feats[rows]),
                "predicts": np.ascontiguousarray(preds[rows]),
                "centers": centers,
            }
        )

    nc = _get_nc()
    res = run_bass_kernel_spmd(nc, in_maps, list(range(N_CORES)))
    partial = np.array(
        [res.results[i]["out"][0, 0] for i in range(N_CORES)], dtype=np.float64
    )
    loss = partial.sum() / N_TOTAL + (NUM_CLASSES - 1) * CLAMP_MIN
    return np.float64(loss)



# revision 38
# speedup vs baseline: 1.1756x; 1.1756x over previous
"""CenterLoss Trainium2 kernel.

Reference computation (see problem statement):
    feats  [N=4096, D=96]  = features.reshape(-1, 96)          (float64 in ref)
    label  [N]             = argmax(predicts, axis=-1)          (fp32 argmax)
    dist_n                 = ||feats_n||^2 + ||c_{l_n}||^2 - 2 feats_n . c_{l_n}
                           = ||feats_n - c_{l_n}||^2
    loss = (sum_n clip(dist_n, 1e-12, 1e12) + (N*C - N) * 1e-12) / N
         -- the (C-1)*1e-12 term comes from clip() lifting the masked-out
            zeros of the [N, C] matrix to 1e-12 each.

Only the labeled column of the [N, C] distance matrix survives the mask, so
the kernel never materializes it: per 128-sample tile it
  1. streams predicts [128, 6625] into SBUF (the dominant cost, ~13.6MB/core),
  2. argmax along the free axis with DVE max / max_index,
  3. indirect-DMA gathers centers[label] rows,
  4. squares (features - gathered) on ACT with accum_out giving the
     per-sample squared distance,
  5. clamps, reduces across partitions with a ones-vector matmul.
Each of the 8 cores handles 512 samples; the host sums the 8 partial sums in
float64 and adds the (C-1)*1e-12 clip constant.
"""

import numpy as np

import concourse.bass as bass
import concourse.mybir as mybir
from concourse import bacc
from concourse.bass_utils import run_bass_kernel_spmd
from concourse.tile import TileContext

NUM_CLASSES = 6625
FEAT_DIM = 96
N_CORES = 8
N_TOTAL = 64 * 64          # 4096 samples
NS = N_TOTAL // N_CORES    # 512 samples per core
P = 128                    # partitions
NTILES = NS // P           # 4 tiles of 128 samples per core
CLAMP_MIN = 1e-12
CLAMP_MAX = 1e12

_NC_CACHE = {}


def _build_nc(
    reps=1,
    pred_bufs=4,
    pass1="hierg",
    dma_only=False,
    fake_gather=False,
    seg_w=128,
    # batching the 4 feature loads into one strided DMA measured ~13 us/rep
    # WORSE (3-level AP descriptor structure); keep per-tile contiguous loads
    batch_feat=False,
    # small_on_act: issue feats loads + out store on the ACT HWDGE ring so the
    # SP ring carries only the 4 big predicts streams per rep
    small_on_act=False,
    # split_pred: split each predicts tile load column-wise across both HWDGE
    # rings (SP gets cols [0, split_at), ACT gets [split_at, 6625))
    split_pred=False,
    split_at=3328,
    # stage: probe decomposition for hierg. 3=full kernel, 2=skip feats
    # load/diff (Square directly on gathered centers), 1=skip centers gather
    # too (copy idx to dacc), 0 equivalent to dma_only.
    stage=3,
    # feat_first: all 4 feats loads into a dedicated double-buffered pool at
    # rep start, so they never stall the SP ring behind small-pool recycling
    feat_first=False,
    # batch_cgather: one 512-row centers indirect gather per rep instead of 4
    batch_cgather=False,
    # out_eng: which engine issues the per-rep scalar out store. "sp" stalls
    # the predicts-stream ring on the whole compute chain; "gp"/"act" don't.
    out_eng="sp",
    # fbufs: >0 gives the per-tile feats tile its own pool with this many
    # buffers, so feats loads on SP never wait on small-pool recycling
    fbufs=0,
    # interleave4: tile j covers samples {4q+j} (partition q <- sample 4q+j),
    # so feats loads as ONE [128, 384] contiguous DMA per rep (128 descriptors
    # of 1536B instead of 512 of 384B). Sum over samples is order-invariant.
    interleave4=False,
    # persist_pred: hierg only -- pred_bufs persistent [P, 6656] buffers whose
    # -1e30 pad columns are filled ONCE at startup (pad never changes), rotated
    # manually across (rep, tile). Removes 4 ACT pad fills per rep and the
    # DMA->pad->reduce dependency.
    persist_pred=False,
    # feat_eng: None=same ring as small_eng; "gp"=SWDGE ring (keeps the SP
    # HWDGE FIFO carrying nothing but the 4 big predicts streams)
    feat_eng=None,
    # batch_cgather2: ONE multi-index centers gather per rep. idx columns are
    # funneled through a single full-tile copy so the gather's descriptor
    # generation depends on ALL four idx writes (sub-range dep race fix).
    batch_cgather2=False,
):
    # seg_w: argmax group width. 128 measured best on HW: narrower groups cut
    # the value-search width but the 3D reduce pays a per-group pipeline
    # restart (208 groups of 32 was ~15us/rep slower than 52 groups of 128).
    # reps>1 repeats the whole per-core computation; used only by the
    # benchmark harness to measure steady-state per-iteration device time
    # as the slope between rep counts (cancels launch + kernel-tail cost).
    # pass1: engine strategy for the argmax -
    #   "dve"  - InstMax + full-width InstMaxIndex (two 1x passes)
    #   "hier" - hierarchical: one reduce_max pass over [128,53,125] ->
    #            group maxes, tiny max_index picks the winning group, an
    #            indirect DMA gathers each row's 125-wide segment, and a
    #            tiny max_index finds the in-group position. DVE cost drops
    #            from two full passes to one.
    # dma_only: benchmark variant that loads predicts but skips the argmax,
    #   to measure the pure DMA floor.
    nc = bacc.Bacc("TRN2", target_bir_lowering=False)
    feats = nc.dram_tensor(
        "features", [NS, FEAT_DIM], mybir.dt.float32, kind="ExternalInput"
    )
    preds = nc.dram_tensor(
        "predicts", [NS, NUM_CLASSES], mybir.dt.float32, kind="ExternalInput"
    )
    cents = nc.dram_tensor(
        "centers", [NUM_CLASSES, FEAT_DIM], mybir.dt.float32, kind="ExternalInput"
    )
    out = nc.dram_tensor("out", [1, 1], mybir.dt.float32, kind="ExternalOutput")

    with TileContext(nc) as tc:
        with (
            tc.tile_pool(name="pred", bufs=pred_bufs) as pred_pool,
            tc.tile_pool(name="small", bufs=3) as small_pool,
            tc.tile_pool(name="fpool", bufs=2) as fpool,
            tc.tile_pool(name="ftp", bufs=max(fbufs, 1)) as ftp,
            tc.tile_pool(name="persist", bufs=1) as persist_pool,
            tc.tile_pool(name="psum", bufs=1, space="PSUM") as psum_pool,
        ):
            small_eng = nc.scalar if small_on_act else nc.sync

            def _load_pred(ptile, rows, j):
                if interleave4:
                    nc.sync.dma_start(
                        out=ptile[:, 0:NUM_CLASSES],
                        in_=preds[:].rearrange("(q f) c -> f q c", f=NTILES)[j],
                    )
                elif split_pred:
                    nc.sync.dma_start(
                        out=ptile[:, 0:split_at], in_=preds[rows, 0:split_at]
                    )
                    nc.scalar.dma_start(
                        out=ptile[:, split_at:NUM_CLASSES],
                        in_=preds[rows, split_at:NUM_CLASSES],
                    )
                else:
                    nc.sync.dma_start(
                        out=ptile[:, 0:NUM_CLASSES], in_=preds[rows, :]
                    )
            ones = persist_pool.tile([P, 1], mybir.dt.float32)
            nc.vector.memset(ones[:], 1.0)

            ptiles = None
            if persist_pred:
                assert pass1 == "hierg"
                ptiles = []
                for i in range(pred_bufs):
                    ppt = persist_pool.tile(
                        [P, 6656], mybir.dt.float32, tag=f"ppt{i}", name=f"ppt{i}"
                    )
                    ptiles.append(ppt)
                for pt in ptiles:
                    nc.scalar.activation(
                        pt[:, NUM_CLASSES:],
                        ones[:, 0:1].to_broadcast([P, 6656 - NUM_CLASSES]),
                        mybir.ActivationFunctionType.Copy,
                        scale=-1e30,
                    )

            G, SEG = 53, 125  # 53 * 125 == 6625
            preds_flat = preds[:].rearrange("n (g k) -> (n g) k", k=SEG)
            if pass1 in ("hier", "hierb"):
                # rowbase[p, j] = (j*128 + p) * G, as fp32 for ACT bias use
                rowbase_i = persist_pool.tile([P, NTILES], mybir.dt.int32)
                nc.gpsimd.iota(
                    rowbase_i[:],
                    pattern=[[P * G, NTILES]],
                    base=0,
                    channel_multiplier=G,
                )
                rowbase_f = persist_pool.tile([P, NTILES], mybir.dt.float32)
                nc.vector.tensor_copy(rowbase_f[:], rowbase_i[:])

            if pass1 == "hierb" and not dma_only:
                for _rep in range(reps):
                    dacc = persist_pool.tile([P, NTILES], mybir.dt.float32, tag="dacc")
                    offs_all = small_pool.tile([P, NTILES], mybir.dt.int32, tag="offs")
                    rmax8s = []
                    g8fs = []
                    # phase A: per-tile group-max + winning group
                    for j in range(NTILES):
                        rows = slice(j * P, (j + 1) * P)
                        ptile = pred_pool.tile([P, NUM_CLASSES], mybir.dt.float32)
                        nc.sync.dma_start(out=ptile[:], in_=preds[rows, :])
                        gmax = small_pool.tile([P, G], mybir.dt.float32)
                        nc.vector.reduce_max(
                            gmax[:],
                            ptile[:].rearrange("p (g k) -> p g k", k=SEG),
                            axis=mybir.AxisListType.X,
                        )
                        rmax = small_pool.tile([P, 1], mybir.dt.float32)
                        nc.vector.reduce_max(
                            rmax[:], gmax[:], axis=mybir.AxisListType.X
                        )
                        rmax8 = small_pool.tile([P, 8], mybir.dt.float32, tag=f"rmax8_{j}")
                        nc.scalar.activation(
                            rmax8[:],
                            rmax[:].to_broadcast([P, 8]),
                            mybir.ActivationFunctionType.Copy,
                        )
                        g8 = small_pool.tile([P, 8], mybir.dt.uint32)
                        nc.vector.max_index(g8[:], rmax8[:], gmax[:])
                        g8f = small_pool.tile([P, 1], mybir.dt.float32, tag=f"g8f_{j}")
                        nc.vector.tensor_copy(g8f[:], g8[:, 0:1])
                        offsf = small_pool.tile([P, 1], mybir.dt.float32)
                        nc.scalar.activation(
                            offsf[:],
                            g8f[:],
                            mybir.ActivationFunctionType.Identity,
                            bias=rowbase_f[:, j : j + 1],
                        )
                        nc.vector.tensor_copy(offs_all[:, j : j + 1], offsf[:])
                        rmax8s.append(rmax8)
                        g8fs.append(g8f)
                    # phase B: one batched segment gather for all 4 tiles
                    seg_all = small_pool.tile([P, NTILES, SEG], mybir.dt.float32)
                    nc.gpsimd.indirect_dma_start(
                        out=seg_all[:],
                        out_offset=None,
                        in_=preds_flat,
                        in_offset=bass.IndirectOffsetOnAxis(
                            ap=offs_all[:, 0:NTILES], axis=0
                        ),
                    )
                    idx_all = small_pool.tile([P, NTILES], mybir.dt.uint32, tag="idxall")
                    for j in range(NTILES):
                        k8 = small_pool.tile([P, 8], mybir.dt.uint32)
                        nc.vector.max_index(k8[:], rmax8s[j][:], seg_all[:, j, :])
                        k8f = small_pool.tile([P, 1], mybir.dt.float32)
                        nc.vector.tensor_copy(k8f[:], k8[:, 0:1])
                        idxf = small_pool.tile([P, 1], mybir.dt.float32)
                        nc.scalar.activation(
                            idxf[:],
                            g8fs[j][:],
                            mybir.ActivationFunctionType.Identity,
                            scale=float(SEG),
                            bias=k8f[:],
                        )
                        nc.vector.tensor_copy(idx_all[:, j : j + 1], idxf[:])
                    # phase C: one batched centers gather + distances
                    call = small_pool.tile([P, NTILES, FEAT_DIM], mybir.dt.float32)
                    nc.gpsimd.indirect_dma_start(
                        out=call[:],
                        out_offset=None,
                        in_=cents[:],
                        in_offset=bass.IndirectOffsetOnAxis(
                            ap=idx_all[:, 0:NTILES], axis=0
                        ),
                    )
                    fall = small_pool.tile([P, NTILES, FEAT_DIM], mybir.dt.float32)
                    nc.sync.dma_start(
                        out=fall[:],
                        in_=feats[:].rearrange("(j p) d -> p j d", p=P),
                    )
                    dall = small_pool.tile([P, NTILES, FEAT_DIM], mybir.dt.float32)
                    nc.vector.tensor_tensor(
                        out=dall[:],
                        in0=fall[:],
                        in1=call[:],
                        op=mybir.AluOpType.subtract,
                    )
                    for j in range(NTILES):
                        sq = small_pool.tile([P, FEAT_DIM], mybir.dt.float32)
                        nc.scalar.activation(
                            sq[:],
                            dall[:, j, :],
                            mybir.ActivationFunctionType.Square,
                            accum_out=dacc[:, j : j + 1],
                        )
                    _final_reduce(nc, persist_pool, psum_pool, dacc, ones, out)

            assert not (batch_cgather and not feat_first), (
                "batch_cgather requires feat_first"
            )
            for _rep in range(reps if pass1 != "hierb" or dma_only else 0):
                dacc = persist_pool.tile([P, NTILES], mybir.dt.float32, tag="dacc")
                fall_t = None
                idxall = None
                if feat_first and not dma_only and stage >= 2:
                    fall_t = fpool.tile(
                        [P, NTILES, FEAT_DIM], mybir.dt.float32, tag="fall2"
                    )
                    for jj in range(NTILES):
                        nc.sync.dma_start(
                            out=fall_t[:, jj, :],
                            in_=feats[jj * P : (jj + 1) * P, :],
                        )
                if batch_cgather and not dma_only:
                    idxall = fpool.tile([P, NTILES], mybir.dt.uint32, tag="idxall")
                if batch_cgather2 and not dma_only:
                    idxall = fpool.tile([P, NTILES], mybir.dt.uint32, tag="idxb")
                    ftl = []
                for j in range(NTILES):
                    rows = slice(j * P, (j + 1) * P)

                    if dma_only:
                        # aligned-pitch tile, same as the hierg data path
                        ptile = pred_pool.tile([P, 6656], mybir.dt.float32)
                        _load_pred(ptile, rows, j)
                        nc.vector.reduce_max(
                            dacc[:, j : j + 1], ptile[:, 0:8],
                            axis=mybir.AxisListType.X,
                        )
                        continue

                    if pass1 == "hierg" and not dma_only:
                        if persist_pred:
                            ptile = ptiles[(_rep * NTILES + j) % pred_bufs]
                            _load_pred(ptile, rows, j)
                        else:
                            # padded to (6656/seg_w) groups x seg_w cols; pad
                            # filled on ACT to keep the vector engine free
                            ptile = pred_pool.tile([P, 6656], mybir.dt.float32)
                            _load_pred(ptile, rows, j)
                            nc.scalar.activation(
                                ptile[:, NUM_CLASSES:],
                                ones[:, 0:1].to_broadcast([P, 6656 - NUM_CLASSES]),
                                mybir.ActivationFunctionType.Copy,
                                scale=-1e30,
                            )
                    else:
                        ptile = pred_pool.tile([P, NUM_CLASSES], mybir.dt.float32)
                        nc.sync.dma_start(out=ptile[:], in_=preds[rows, :])

                    if dma_only:
                        # touch a sliver so the load isn't dead
                        nc.vector.reduce_max(
                            dacc[:, j : j + 1], ptile[:, 0:8],
                            axis=mybir.AxisListType.X,
                        )
                        continue

                    if pass1 == "hierg":
                        # groups of seg_w columns (padded with -1e30 so class
                        # index == g*seg_w + k, recoverable with bit ops)
                        SP = seg_w
                        GP = 6656 // SP
                        SHIFT = SP.bit_length() - 1
                        assert 1 << SHIFT == SP
                        gmax = small_pool.tile([P, GP], mybir.dt.float32)
                        nc.vector.reduce_max(
                            gmax[:],
                            ptile[:].rearrange("p (g k) -> p g k", k=SP),
                            axis=mybir.AxisListType.X,
                        )
                        # top-8 group maxes, sorted desc: slot 0 is the row
                        # max, and the tile doubles as max_index's in_max
                        rmax8 = small_pool.tile([P, 8], mybir.dt.float32)
                        nc.vector.max(rmax8[:], gmax[:])
                        g8 = small_pool.tile([P, 8], mybir.dt.uint16)
                        nc.vector.max_index(g8[:], rmax8[:], gmax[:])
                        # gather each partition's winning 128-wide group from
                        # SBUF on GpSimd. ap_gather broadcasts each of the 16
                        # partition indices to the whole 16-partition group, so
                        # partition p's own winning group lands at block p%16;
                        # the row max value only occurs in p's own group (any
                        # duplicate block is that same group, with the value at
                        # the same in-block position), so a value search over
                        # all 16 blocks yields the right in-group position.
                        blocks = small_pool.tile([P, 16, SP], mybir.dt.float32)
                        nc.gpsimd.ap_gather(
                            out_ap=blocks[:],
                            in_ap=ptile[:].rearrange("p (g k) -> p g k", k=SP),
                            idxs_ap=g8[:, 0:1].bitcast(mybir.dt.int16),
                            channels=P,
                            num_elems=GP,
                            d=SP,
                            num_idxs=16,
                        )
                        f8 = small_pool.tile([P, 8], mybir.dt.uint16)
                        nc.vector.max_index(
                            f8[:],
                            rmax8[:],
                            blocks[:].rearrange("p a b -> p (a b)"),
                        )
                        # class index = (g << SHIFT) + (found & (SP-1))
                        kmod = small_pool.tile([P, 1], mybir.dt.uint16)
                        nc.vector.tensor_scalar(
                            out=kmod[:],
                            in0=f8[:, 0:1],
                            scalar1=SP - 1,
                            scalar2=None,
                            op0=mybir.AluOpType.bitwise_and,
                        )
                        gshl = small_pool.tile([P, 1], mybir.dt.uint16)
                        nc.vector.tensor_scalar(
                            out=gshl[:],
                            in0=g8[:, 0:1],
                            scalar1=SHIFT,
                            scalar2=None,
                            op0=mybir.AluOpType.logical_shift_left,
                        )
                        idx8 = small_pool.tile([P, 1], mybir.dt.uint32)
                        nc.vector.tensor_tensor(
                            out=idx8[:],
                            in0=gshl[:],
                            in1=kmod[:],
                            op=mybir.AluOpType.add,
                        )
                    elif pass1 == "hier":
                        gmax = small_pool.tile([P, G], mybir.dt.float32)
                        nc.vector.reduce_max(
                            gmax[:],
                            ptile[:].rearrange("p (g k) -> p g k", k=SEG),
                            axis=mybir.AxisListType.X,
                        )
                        rmax = small_pool.tile([P, 1], mybir.dt.float32)
                        nc.vector.reduce_max(
                            rmax[:], gmax[:], axis=mybir.AxisListType.X
                        )
                        rmax8 = small_pool.tile([P, 8], mybir.dt.float32)
                        nc.scalar.activation(
                            rmax8[:],
                            rmax[:].to_broadcast([P, 8]),
                            mybir.ActivationFunctionType.Copy,
                        )
                        g8 = small_pool.tile([P, 8], mybir.dt.uint32)
                        nc.vector.max_index(g8[:], rmax8[:], gmax[:])
                        g8f = small_pool.tile([P, 1], mybir.dt.float32)
                        nc.vector.tensor_copy(g8f[:], g8[:, 0:1])
                        # offset into preds_flat: row*G + g
                        offsf = small_pool.tile([P, 1], mybir.dt.float32)
                        nc.scalar.activation(
                            offsf[:],
                            g8f[:],
                            mybir.ActivationFunctionType.Identity,
                            bias=rowbase_f[:, j : j + 1],
                        )
                        offsi = small_pool.tile([P, 1], mybir.dt.int32)
                        nc.vector.tensor_copy(offsi[:], offsf[:])
                        seg = small_pool.tile([P, SEG], mybir.dt.float32)
                        if fake_gather:
                            # benchmark probe: fixed-window read instead of a
                            # per-row indirect gather (wrong results)
                            nc.vector.tensor_copy(seg[:], ptile[:, 0:SEG])
                        else:
                            nc.gpsimd.indirect_dma_start(
                                out=seg[:],
                                out_offset=None,
                                in_=preds_flat,
                                in_offset=bass.IndirectOffsetOnAxis(
                                    ap=offsi[:, 0:1], axis=0
                                ),
                            )
                        k8 = small_pool.tile([P, 8], mybir.dt.uint32)
                        nc.vector.max_index(k8[:], rmax8[:], seg[:])
                        k8f = small_pool.tile([P, 1], mybir.dt.float32)
                        nc.vector.tensor_copy(k8f[:], k8[:, 0:1])
                        # class index: g*SEG + k
                        idxf = small_pool.tile([P, 1], mybir.dt.float32)
                        nc.scalar.activation(
                            idxf[:],
                            g8f[:],
                            mybir.ActivationFunctionType.Identity,
                            scale=float(SEG),
                            bias=k8f[:],
                        )
                        idx8 = small_pool.tile([P, 1], mybir.dt.uint32)
                        nc.vector.tensor_copy(idx8[:], idxf[:])
                    else:
                        max8 = small_pool.tile([P, 8], mybir.dt.float32)
                        idx8 = small_pool.tile([P, 8], mybir.dt.uint32)
                        nc.vector.max(max8[:], ptile[:])
                        nc.vector.max_index(idx8[:], max8[:], ptile[:])

                    if stage <= 1:
                        # probe: argmax only; fold idx into dacc so it's live
                        nc.vector.tensor_copy(dacc[:, j : j + 1], idx8[:, 0:1])
                        continue

                    if batch_cgather:
                        nc.vector.tensor_copy(idxall[:, j : j + 1], idx8[:, 0:1])
                        continue

                    if batch_cgather2:
                        nc.vector.tensor_copy(idxall[:, j : j + 1], idx8[:, 0:1])
                        ftile_t = ftp.tile(
                            [P, FEAT_DIM], mybir.dt.float32, tag=f"ft{j}"
                        )
                        feng = (
                            {"gp": nc.gpsimd, "act": nc.scalar}[feat_eng]
                            if feat_eng
                            else small_eng
                        )
                        feng.dma_start(out=ftile_t[:], in_=feats[rows, :])
                        ftl.append(ftile_t)
                        continue

                    if pass1 == "hierg" and batch_feat:
                        # all 512 feature rows in one strided DMA per rep
                        if j == 0:
                            fall = small_pool.tile(
                                [P, NTILES, FEAT_DIM], mybir.dt.float32, tag="fall"
                            )
                            nc.sync.dma_start(
                                out=fall[:],
                                in_=feats[:].rearrange("(j p) d -> p j d", p=P),
                            )
                        ftile = fall[:, j, :]
                    elif feat_first:
                        ftile = fall_t[:, j, :]
                    elif interleave4:
                        if j == 0:
                            fall4 = ftp.tile(
                                [P, NTILES * FEAT_DIM], mybir.dt.float32, tag="f4"
                            )
                            nc.sync.dma_start(
                                out=fall4[:],
                                in_=feats[:].rearrange(
                                    "(q f) d -> q (f d)", f=NTILES
                                ),
                            )
                        ftile = fall4[:, j * FEAT_DIM : (j + 1) * FEAT_DIM]
                    elif stage >= 3:
                        fp = ftp if fbufs > 0 else small_pool
                        ftile_t = fp.tile([P, FEAT_DIM], mybir.dt.float32)
                        feng = (
                            {"gp": nc.gpsimd, "act": nc.scalar}[feat_eng]
                            if feat_eng
                            else small_eng
                        )
                        feng.dma_start(out=ftile_t[:], in_=feats[rows, :])
                        ftile = ftile_t[:]

                    ctile = small_pool.tile([P, FEAT_DIM], mybir.dt.float32)
                    if fake_gather:
                        nc.sync.dma_start(out=ctile[:], in_=cents[0:P, :])
                    else:
                        nc.gpsimd.indirect_dma_start(
                            out=ctile[:],
                            out_offset=None,
                            in_=cents[:],
                            in_offset=bass.IndirectOffsetOnAxis(
                                ap=idx8[:, 0:1], axis=0
                            ),
                        )

                    if stage == 2:
                        # probe: skip feats load/diff, square the gathered rows
                        sq = small_pool.tile([P, FEAT_DIM], mybir.dt.float32)
                        nc.scalar.activation(
                            sq[:],
                            ctile[:],
                            mybir.ActivationFunctionType.Square,
                            accum_out=dacc[:, j : j + 1],
                        )
                        continue

                    diff = small_pool.tile([P, FEAT_DIM], mybir.dt.float32)
                    nc.vector.tensor_tensor(
                        out=diff[:],
                        in0=ftile,
                        in1=ctile[:],
                        op=mybir.AluOpType.subtract,
                    )
                    sq = small_pool.tile([P, FEAT_DIM], mybir.dt.float32)
                    nc.scalar.activation(
                        sq[:],
                        diff[:],
                        mybir.ActivationFunctionType.Square,
                        accum_out=dacc[:, j : j + 1],
                    )

                if batch_cgather2 and not dma_only:
                    idxall2 = fpool.tile([P, NTILES], mybir.dt.uint32, tag="idxb2")
                    nc.vector.tensor_copy(idxall2[:], idxall[:])
                    callb = fpool.tile(
                        [P, NTILES, FEAT_DIM], mybir.dt.float32, tag="callb2"
                    )
                    nc.gpsimd.indirect_dma_start(
                        out=callb[:],
                        out_offset=None,
                        in_=cents[:],
                        in_offset=bass.IndirectOffsetOnAxis(
                            ap=idxall2[:, 0:NTILES], axis=0
                        ),
                    )
                    for j in range(NTILES):
                        diff = small_pool.tile([P, FEAT_DIM], mybir.dt.float32)
                        nc.vector.tensor_tensor(
                            out=diff[:],
                            in0=ftl[j][:],
                            in1=callb[:, j, :],
                            op=mybir.AluOpType.subtract,
                        )
                        sq = small_pool.tile([P, FEAT_DIM], mybir.dt.float32)
                        nc.scalar.activation(
                            sq[:],
                            diff[:],
                            mybir.ActivationFunctionType.Square,
                            accum_out=dacc[:, j : j + 1],
                        )
                if batch_cgather and not dma_only:
                    call = fpool.tile(
                        [P, NTILES, FEAT_DIM], mybir.dt.float32, tag="call2"
                    )
                    nc.gpsimd.indirect_dma_start(
                        out=call[:],
                        out_offset=None,
                        in_=cents[:],
                        in_offset=bass.IndirectOffsetOnAxis(
                            ap=idxall[:, 0:NTILES], axis=0
                        ),
                    )
                    for j in range(NTILES):
                        diff = small_pool.tile([P, FEAT_DIM], mybir.dt.float32)
                        nc.vector.tensor_tensor(
                            out=diff[:],
                            in0=fall_t[:, j, :],
                            in1=call[:, j, :],
                            op=mybir.AluOpType.subtract,
                        )
                        sq = small_pool.tile([P, FEAT_DIM], mybir.dt.float32)
                        nc.scalar.activation(
                            sq[:],
                            diff[:],
                            mybir.ActivationFunctionType.Square,
                            accum_out=dacc[:, j : j + 1],
                        )
                oeng = {"sp": nc.sync, "act": nc.scalar, "gp": nc.gpsimd}[out_eng]
                _final_reduce(
                    nc, persist_pool, psum_pool, dacc, ones, out, eng=oeng
                )

    nc.compile()
    return nc


def _build_nc_v2(
    reps=1,
    npair_bufs=3,
    out_eng="gp",
    cg_batch=False,
):
    """Pair-load layout: tile j covers samples {4q+j} (partition q). Tiles
    {0,1} and {2,3} are adjacent DRAM rows per partition, so each rep does
    just 2 big predicts DMAs ([P, 2, 6625], ~53KB/partition) + 1 contiguous
    feats DMA. Pad columns are persistent (filled once). Argmax per sub-tile
    is the hierg chain (group reduce_max + top8 + ap_gather + bit ops)."""
    nc = bacc.Bacc("TRN2", target_bir_lowering=False)
    feats = nc.dram_tensor(
        "features", [NS, FEAT_DIM], mybir.dt.float32, kind="ExternalInput"
    )
    preds = nc.dram_tensor(
        "predicts", [NS, NUM_CLASSES], mybir.dt.float32, kind="ExternalInput"
    )
    cents = nc.dram_tensor(
        "centers", [NUM_CLASSES, FEAT_DIM], mybir.dt.float32, kind="ExternalInput"
    )
    out = nc.dram_tensor("out", [1, 1], mybir.dt.float32, kind="ExternalOutput")

    SP = 128
    GP = 6656 // SP  # 52 groups
    SHIFT = SP.bit_length() - 1

    with TileContext(nc) as tc:
        with (
            tc.tile_pool(name="small", bufs=3) as small_pool,
            tc.tile_pool(name="ftp", bufs=2) as ftp,
            tc.tile_pool(name="persist", bufs=1) as persist_pool,
            tc.tile_pool(name="psum", bufs=1, space="PSUM") as psum_pool,
        ):
            ones = persist_pool.tile([P, 1], mybir.dt.float32)
            nc.vector.memset(ones[:], 1.0)

            oeng = {"sp": nc.sync, "act": nc.scalar, "gp": nc.gpsimd}[out_eng]

            pts = []
            for i in range(npair_bufs):
                ppt = persist_pool.tile(
                    [P, 2, 6656], mybir.dt.float32, tag=f"pp{i}", name=f"pp{i}"
                )
                for t in range(2):
                    nc.scalar.activation(
                        ppt[:, t, NUM_CLASSES:],
                        ones[:, 0:1].to_broadcast([P, 6656 - NUM_CLASSES]),
                        mybir.ActivationFunctionType.Copy,
                        scale=-1e30,
                    )
                pts.append(ppt)

            preds4 = preds[:].rearrange("(q f) c -> q f c", f=NTILES)

            for _rep in range(reps):
                dacc = persist_pool.tile([P, NTILES], mybir.dt.float32, tag="dacc")
                fall4 = ftp.tile([P, NTILES * FEAT_DIM], mybir.dt.float32)
                nc.sync.dma_start(
                    out=fall4[:],
                    in_=feats[:].rearrange("(q f) d -> q (f d)", f=NTILES),
                )
                idxall = None
                if cg_batch:
                    idxall = ftp.tile([P, NTILES], mybir.dt.uint32, tag="idxa")
                for pair in range(2):
                    ppt = pts[(_rep * 2 + pair) % npair_bufs]
                    nc.sync.dma_start(
                        out=ppt[:, :, 0:NUM_CLASSES],
                        in_=preds4[:, 2 * pair : 2 * pair + 2, :],
                    )
                    for t in range(2):
                        j = 2 * pair + t
                        ptv = ppt[:, t, :]
                        ptv3 = ppt[:, t, :].rearrange("p (g k) -> p g k", k=SP)
                        gmax = small_pool.tile([P, GP], mybir.dt.float32)
                        nc.vector.reduce_max(
                            gmax[:], ptv3, axis=mybir.AxisListType.X
                        )
                        rmax8 = small_pool.tile([P, 8], mybir.dt.float32)
                        nc.vector.max(rmax8[:], gmax[:])
                        g8 = small_pool.tile([P, 8], mybir.dt.uint16)
                        nc.vector.max_index(g8[:], rmax8[:], gmax[:])
                        blocks = small_pool.tile([P, 16, SP], mybir.dt.float32)
                        nc.gpsimd.ap_gather(
                            out_ap=blocks[:],
                            in_ap=ptv3,
                            idxs_ap=g8[:, 0:1].bitcast(mybir.dt.int16),
                            channels=P,
                            num_elems=GP,
                            d=SP,
                            num_idxs=16,
                        )
                        f8 = small_pool.tile([P, 8], mybir.dt.uint16)
                        nc.vector.max_index(
                            f8[:],
                            rmax8[:],
                            blocks[:].rearrange("p a b -> p (a b)"),
                        )
                        kmod = small_pool.tile([P, 1], mybir.dt.uint16)
                        nc.vector.tensor_scalar(
                            out=kmod[:],
                            in0=f8[:, 0:1],
                            scalar1=SP - 1,
                            scalar2=None,
                            op0=mybir.AluOpType.bitwise_and,
                        )
                        gshl = small_pool.tile([P, 1], mybir.dt.uint16)
                        nc.vector.tensor_scalar(
                            out=gshl[:],
                            in0=g8[:, 0:1],
                            scalar1=SHIFT,
                            scalar2=None,
                            op0=mybir.AluOpType.logical_shift_left,
                        )
                        idx8 = small_pool.tile([P, 1], mybir.dt.uint32)
                        nc.vector.tensor_tensor(
                            out=idx8[:],
                            in0=gshl[:],
                            in1=kmod[:],
                            op=mybir.AluOpType.add,
                        )
                        if cg_batch:
                            nc.vector.tensor_copy(
                                idxall[:, j : j + 1], idx8[:, 0:1]
                            )
                            continue
                        ctile = small_pool.tile([P, FEAT_DIM], mybir.dt.float32)
                        nc.gpsimd.indirect_dma_start(
                            out=ctile[:],
                            out_offset=None,
                            in_=cents[:],
                            in_offset=bass.IndirectOffsetOnAxis(
                                ap=idx8[:, 0:1], axis=0
                            ),
                        )
                        diff = small_pool.tile([P, FEAT_DIM], mybir.dt.float32)
                        nc.vector.tensor_tensor(
                            out=diff[:],
                            in0=fall4[:, j * FEAT_DIM : (j + 1) * FEAT_DIM],
                            in1=ctile[:],
                            op=mybir.AluOpType.subtract,
                        )
                        sq = small_pool.tile([P, FEAT_DIM], mybir.dt.float32)
                        nc.scalar.activation(
                            sq[:],
                            diff[:],
                            mybir.ActivationFunctionType.Square,
                            accum_out=dacc[:, j : j + 1],
                        )
                if cg_batch:
                    call = ftp.tile(
                        [P, NTILES, FEAT_DIM], mybir.dt.float32, tag="callb"
                    )
                    nc.gpsimd.indirect_dma_start(
                        out=call[:],
                        out_offset=None,
                        in_=cents[:],
                        in_offset=bass.IndirectOffsetOnAxis(
                            ap=idxall[:, 0:NTILES], axis=0
                        ),
                    )
                    for j in range(NTILES):
                        diff = small_pool.tile([P, FEAT_DIM], mybir.dt.float32)
                        nc.vector.tensor_tensor(
                            out=diff[:],
                            in0=fall4[:, j * FEAT_DIM : (j + 1) * FEAT_DIM],
                            in1=call[:, j, :],
                            op=mybir.AluOpType.subtract,
                        )
                        sq = small_pool.tile([P, FEAT_DIM], mybir.dt.float32)
                        nc.scalar.activation(
                            sq[:],
                            diff[:],
                            mybir.ActivationFunctionType.Square,
                            accum_out=dacc[:, j : j + 1],
                        )
                _final_reduce(nc, persist_pool, psum_pool, dacc, ones, out, eng=oeng)

    nc.compile()
    return nc


def _final_reduce(nc, persist_pool, psum_pool, dacc, ones, out, eng=None):
    dclamp = persist_pool.tile([P, NTILES], mybir.dt.float32, tag="dclamp")
    nc.vector.tensor_scalar(
        out=dclamp[:],
        in0=dacc[:],
        scalar1=CLAMP_MIN,
        scalar2=CLAMP_MAX,
        op0=mybir.AluOpType.max,
        op1=mybir.AluOpType.min,
    )
    dsum = persist_pool.tile([P, 1], mybir.dt.float32, tag="dsum")
    nc.vector.reduce_sum(dsum[:], dclamp[:], axis=mybir.AxisListType.X)

    res_psum = psum_pool.tile([1, 1], mybir.dt.float32)
    nc.tensor.matmul(res_psum[:], lhsT=dsum[:], rhs=ones[:], start=True, stop=True)
    res_sb = persist_pool.tile([1, 1], mybir.dt.float32, tag="res_sb")
    nc.vector.tensor_copy(res_sb[:], res_psum[:])
    (eng or nc.sync).dma_start(out=out[:], in_=res_sb[:])


# Best measured configuration (interleaved A/B, 100 trials): single
# contiguous feats DMA (interleave4 sample mapping), out store on the SWDGE
# queue so it never stalls the SP predicts stream, persistent predicts
# buffers with one-time -1e30 pad fill.
BEST_KWARGS = dict(
    interleave4=True, fbufs=2, out_eng="gp", persist_pred=True, pred_bufs=4
)


def _get_nc():
    if "nc" not in _NC_CACHE:
        _NC_CACHE["nc"] = _build_nc(**BEST_KWARGS)
    return _NC_CACHE["nc"]


def kernel(features, predicts, centers):
    features = np.ascontiguousarray(np.asarray(features, dtype=np.float32))
    predicts = np.ascontiguousarray(np.asarray(predicts, dtype=np.float32))
    centers = np.ascontiguousarray(np.asarray(centers, dtype=np.float32))

    feats = features.reshape(N_TOTAL, FEAT_DIM)
    preds = predicts.reshape(N_TOTAL, NUM_CLASSES)

    in_maps = []
    for c in range(N_CORES):
        rows = slice(c * NS, (c + 1) * NS)
        in_maps.append(
            {
                "features": np.ascontiguousarray(feats[rows]),
                "predicts": np.ascontiguousarray(preds[rows]),
                "centers": centers,
            }
        )

    nc = _get_nc()
    res = run_bass_kernel_spmd(nc, in_maps, list(range(N_CORES)))
    partial = np.array(
        [res.results[i]["out"][0, 0] for i in range(N_CORES)], dtype=np.float64
    )
    loss = partial.sum() / N_TOTAL + (NUM_CLASSES - 1) * CLAMP_MIN
    return np.float64(loss)



# revision 43
# speedup vs baseline: 1.3275x; 1.1292x over previous
"""CenterLoss Trainium2 kernel.

Reference computation (see problem statement):
    feats  [N=4096, D=96]  = features.reshape(-1, 96)          (float64 in ref)
    label  [N]             = argmax(predicts, axis=-1)          (fp32 argmax)
    dist_n                 = ||feats_n||^2 + ||c_{l_n}||^2 - 2 feats_n . c_{l_n}
                           = ||feats_n - c_{l_n}||^2
    loss = (sum_n clip(dist_n, 1e-12, 1e12) + (N*C - N) * 1e-12) / N
         -- the (C-1)*1e-12 term comes from clip() lifting the masked-out
            zeros of the [N, C] matrix to 1e-12 each.

Only the labeled column of the [N, C] distance matrix survives the mask, so
the kernel never materializes it: per 128-sample tile it
  1. streams predicts [128, 6625] into SBUF (the dominant cost, ~13.6MB/core),
  2. argmax along the free axis with DVE max / max_index,
  3. indirect-DMA gathers centers[label] rows,
  4. squares (features - gathered) on ACT with accum_out giving the
     per-sample squared distance,
  5. clamps, reduces across partitions with a ones-vector matmul.
Each of the 8 cores handles 512 samples; the host sums the 8 partial sums in
float64 and adds the (C-1)*1e-12 clip constant.
"""

import numpy as np

import concourse.bass as bass
import concourse.mybir as mybir
from concourse import bacc
from concourse.bass_utils import run_bass_kernel_spmd
from concourse.tile import TileContext

NUM_CLASSES = 6625
FEAT_DIM = 96
N_CORES = 8
N_TOTAL = 64 * 64          # 4096 samples
NS = N_TOTAL // N_CORES    # 512 samples per core
P = 128                    # partitions
NTILES = NS // P           # 4 tiles of 128 samples per core
CLAMP_MIN = 1e-12
CLAMP_MAX = 1e12

_NC_CACHE = {}


def _build_nc(
    reps=1,
    pred_bufs=4,
    pass1="hierg",
    dma_only=False,
    fake_gather=False,
    seg_w=128,
    # batching the 4 feature loads into one strided DMA measured ~13 us/rep
    # WORSE (3-level AP descriptor structure); keep per-tile contiguous loads
    batch_feat=False,
    # small_on_act: issue feats loads + out store on the ACT HWDGE ring so the
    # SP ring carries only the 4 big predicts streams per rep
    small_on_act=False,
    # split_pred: split each predicts tile load column-wise across both HWDGE
    # rings (SP gets cols [0, split_at), ACT gets [split_at, 6625))
    split_pred=False,
    split_at=3328,
    # stage: probe decomposition for hierg. 3=full kernel, 2=skip feats
    # load/diff (Square directly on gathered centers), 1=skip centers gather
    # too (copy idx to dacc), 0 equivalent to dma_only.
    stage=3,
    # feat_first: all 4 feats loads into a dedicated double-buffered pool at
    # rep start, so they never stall the SP ring behind small-pool recycling
    feat_first=False,
    # batch_cgather: one 512-row centers indirect gather per rep instead of 4
    batch_cgather=False,
    # out_eng: which engine issues the per-rep scalar out store. "sp" stalls
    # the predicts-stream ring on the whole compute chain; "gp"/"act" don't.
    out_eng="sp",
    # fbufs: >0 gives the per-tile feats tile its own pool with this many
    # buffers, so feats loads on SP never wait on small-pool recycling
    fbufs=0,
    # interleave4: tile j covers samples {4q+j} (partition q <- sample 4q+j),
    # so feats loads as ONE [128, 384] contiguous DMA per rep (128 descriptors
    # of 1536B instead of 512 of 384B). Sum over samples is order-invariant.
    interleave4=False,
    # persist_pred: hierg only -- pred_bufs persistent [P, 6656] buffers whose
    # -1e30 pad columns are filled ONCE at startup (pad never changes), rotated
    # manually across (rep, tile). Removes 4 ACT pad fills per rep and the
    # DMA->pad->reduce dependency.
    persist_pred=False,
    # feat_eng: None=same ring as small_eng; "gp"=SWDGE ring (keeps the SP
    # HWDGE FIFO carrying nothing but the 4 big predicts streams)
    feat_eng=None,
    # batch_cgather2: ONE multi-index centers gather per rep. idx columns are
    # funneled through a single full-tile copy so the gather's descriptor
    # generation depends on ALL four idx writes (sub-range dep race fix).
    batch_cgather2=False,
    # fuse_gather: requires interleave4. Pre-fill the gather destination with
    # -x_j (ACT copy, early), then indirect-gather with compute_op=add so the
    # CCE lands c - x directly: the DVE diff leaves the gather->square chain.
    fuse_gather=False,
):
    # seg_w: argmax group width. 128 measured best on HW: narrower groups cut
    # the value-search width but the 3D reduce pays a per-group pipeline
    # restart (208 groups of 32 was ~15us/rep slower than 52 groups of 128).
    # reps>1 repeats the whole per-core computation; used only by the
    # benchmark harness to measure steady-state per-iteration device time
    # as the slope between rep counts (cancels launch + kernel-tail cost).
    # pass1: engine strategy for the argmax -
    #   "dve"  - InstMax + full-width InstMaxIndex (two 1x passes)
    #   "hier" - hierarchical: one reduce_max pass over [128,53,125] ->
    #            group maxes, tiny max_index picks the winning group, an
    #            indirect DMA gathers each row's 125-wide segment, and a
    #            tiny max_index finds the in-group position. DVE cost drops
    #            from two full passes to one.
    # dma_only: benchmark variant that loads predicts but skips the argmax,
    #   to measure the pure DMA floor.
    nc = bacc.Bacc("TRN2", target_bir_lowering=False)
    feats = nc.dram_tensor(
        "features", [NS, FEAT_DIM], mybir.dt.float32, kind="ExternalInput"
    )
    preds = nc.dram_tensor(
        "predicts", [NS, NUM_CLASSES], mybir.dt.float32, kind="ExternalInput"
    )
    cents = nc.dram_tensor(
        "centers", [NUM_CLASSES, FEAT_DIM], mybir.dt.float32, kind="ExternalInput"
    )
    out = nc.dram_tensor("out", [1, 1], mybir.dt.float32, kind="ExternalOutput")

    with TileContext(nc) as tc:
        with (
            tc.tile_pool(name="pred", bufs=pred_bufs) as pred_pool,
            tc.tile_pool(name="small", bufs=3) as small_pool,
            tc.tile_pool(name="fpool", bufs=2) as fpool,
            tc.tile_pool(name="ftp", bufs=max(fbufs, 1)) as ftp,
            tc.tile_pool(name="persist", bufs=1) as persist_pool,
            tc.tile_pool(name="psum", bufs=1, space="PSUM") as psum_pool,
        ):
            small_eng = nc.scalar if small_on_act else nc.sync

            def _load_pred(ptile, rows, j):
                if interleave4:
                    nc.sync.dma_start(
                        out=ptile[:, 0:NUM_CLASSES],
                        in_=preds[:].rearrange("(q f) c -> f q c", f=NTILES)[j],
                    )
                elif split_pred:
                    nc.sync.dma_start(
                        out=ptile[:, 0:split_at], in_=preds[rows, 0:split_at]
                    )
                    nc.scalar.dma_start(
                        out=ptile[:, split_at:NUM_CLASSES],
                        in_=preds[rows, split_at:NUM_CLASSES],
                    )
                else:
                    nc.sync.dma_start(
                        out=ptile[:, 0:NUM_CLASSES], in_=preds[rows, :]
                    )
            ones = persist_pool.tile([P, 1], mybir.dt.float32)
            nc.vector.memset(ones[:], 1.0)

            ptiles = None
            if persist_pred:
                assert pass1 == "hierg"
                ptiles = []
                for i in range(pred_bufs):
                    ppt = persist_pool.tile(
                        [P, 6656], mybir.dt.float32, tag=f"ppt{i}", name=f"ppt{i}"
                    )
                    ptiles.append(ppt)
                for pt in ptiles:
                    nc.scalar.activation(
                        pt[:, NUM_CLASSES:],
                        ones[:, 0:1].to_broadcast([P, 6656 - NUM_CLASSES]),
                        mybir.ActivationFunctionType.Copy,
                        scale=-1e30,
                    )

            G, SEG = 53, 125  # 53 * 125 == 6625
            preds_flat = preds[:].rearrange("n (g k) -> (n g) k", k=SEG)
            if pass1 in ("hier", "hierb"):
                # rowbase[p, j] = (j*128 + p) * G, as fp32 for ACT bias use
                rowbase_i = persist_pool.tile([P, NTILES], mybir.dt.int32)
                nc.gpsimd.iota(
                    rowbase_i[:],
                    pattern=[[P * G, NTILES]],
                    base=0,
                    channel_multiplier=G,
                )
                rowbase_f = persist_pool.tile([P, NTILES], mybir.dt.float32)
                nc.vector.tensor_copy(rowbase_f[:], rowbase_i[:])

            if pass1 == "hierb" and not dma_only:
                for _rep in range(reps):
                    dacc = persist_pool.tile([P, NTILES], mybir.dt.float32, tag="dacc")
                    offs_all = small_pool.tile([P, NTILES], mybir.dt.int32, tag="offs")
                    rmax8s = []
                    g8fs = []
                    # phase A: per-tile group-max + winning group
                    for j in range(NTILES):
                        rows = slice(j * P, (j + 1) * P)
                        ptile = pred_pool.tile([P, NUM_CLASSES], mybir.dt.float32)
                        nc.sync.dma_start(out=ptile[:], in_=preds[rows, :])
                        gmax = small_pool.tile([P, G], mybir.dt.float32)
                        nc.vector.reduce_max(
                            gmax[:],
                            ptile[:].rearrange("p (g k) -> p g k", k=SEG),
                            axis=mybir.AxisListType.X,
                        )
                        rmax = small_pool.tile([P, 1], mybir.dt.float32)
                        nc.vector.reduce_max(
                            rmax[:], gmax[:], axis=mybir.AxisListType.X
                        )
                        rmax8 = small_pool.tile([P, 8], mybir.dt.float32, tag=f"rmax8_{j}")
                        nc.scalar.activation(
                            rmax8[:],
                            rmax[:].to_broadcast([P, 8]),
                            mybir.ActivationFunctionType.Copy,
                        )
                        g8 = small_pool.tile([P, 8], mybir.dt.uint32)
                        nc.vector.max_index(g8[:], rmax8[:], gmax[:])
                        g8f = small_pool.tile([P, 1], mybir.dt.float32, tag=f"g8f_{j}")
                        nc.vector.tensor_copy(g8f[:], g8[:, 0:1])
                        offsf = small_pool.tile([P, 1], mybir.dt.float32)
                        nc.scalar.activation(
                            offsf[:],
                            g8f[:],
                            mybir.ActivationFunctionType.Identity,
                            bias=rowbase_f[:, j : j + 1],
                        )
                        nc.vector.tensor_copy(offs_all[:, j : j + 1], offsf[:])
                        rmax8s.append(rmax8)
                        g8fs.append(g8f)
                    # phase B: one batched segment gather for all 4 tiles
                    seg_all = small_pool.tile([P, NTILES, SEG], mybir.dt.float32)
                    nc.gpsimd.indirect_dma_start(
                        out=seg_all[:],
                        out_offset=None,
                        in_=preds_flat,
                        in_offset=bass.IndirectOffsetOnAxis(
                            ap=offs_all[:, 0:NTILES], axis=0
                        ),
                    )
                    idx_all = small_pool.tile([P, NTILES], mybir.dt.uint32, tag="idxall")
                    for j in range(NTILES):
                        k8 = small_pool.tile([P, 8], mybir.dt.uint32)
                        nc.vector.max_index(k8[:], rmax8s[j][:], seg_all[:, j, :])
                        k8f = small_pool.tile([P, 1], mybir.dt.float32)
                        nc.vector.tensor_copy(k8f[:], k8[:, 0:1])
                        idxf = small_pool.tile([P, 1], mybir.dt.float32)
                        nc.scalar.activation(
                            idxf[:],
                            g8fs[j][:],
                            mybir.ActivationFunctionType.Identity,
                            scale=float(SEG),
                            bias=k8f[:],
                        )
                        nc.vector.tensor_copy(idx_all[:, j : j + 1], idxf[:])
                    # phase C: one batched centers gather + distances
                    call = small_pool.tile([P, NTILES, FEAT_DIM], mybir.dt.float32)
                    nc.gpsimd.indirect_dma_start(
                        out=call[:],
                        out_offset=None,
                        in_=cents[:],
                        in_offset=bass.IndirectOffsetOnAxis(
                            ap=idx_all[:, 0:NTILES], axis=0
                        ),
                    )
                    fall = small_pool.tile([P, NTILES, FEAT_DIM], mybir.dt.float32)
                    nc.sync.dma_start(
                        out=fall[:],
                        in_=feats[:].rearrange("(j p) d -> p j d", p=P),
                    )
                    dall = small_pool.tile([P, NTILES, FEAT_DIM], mybir.dt.float32)
                    nc.vector.tensor_tensor(
                        out=dall[:],
                        in0=fall[:],
                        in1=call[:],
                        op=mybir.AluOpType.subtract,
                    )
                    for j in range(NTILES):
                        sq = small_pool.tile([P, FEAT_DIM], mybir.dt.float32)
                        nc.scalar.activation(
                            sq[:],
                            dall[:, j, :],
                            mybir.ActivationFunctionType.Square,
                            accum_out=dacc[:, j : j + 1],
                        )
                    _final_reduce(nc, persist_pool, psum_pool, dacc, ones, out)

            assert not (batch_cgather and not feat_first), (
                "batch_cgather requires feat_first"
            )
            for _rep in range(reps if pass1 != "hierb" or dma_only else 0):
                dacc = persist_pool.tile([P, NTILES], mybir.dt.float32, tag="dacc")
                fall_t = None
                idxall = None
                if feat_first and not dma_only and stage >= 2:
                    fall_t = fpool.tile(
                        [P, NTILES, FEAT_DIM], mybir.dt.float32, tag="fall2"
                    )
                    for jj in range(NTILES):
                        nc.sync.dma_start(
                            out=fall_t[:, jj, :],
                            in_=feats[jj * P : (jj + 1) * P, :],
                        )
                if batch_cgather and not dma_only:
                    idxall = fpool.tile([P, NTILES], mybir.dt.uint32, tag="idxall")
                if batch_cgather2 and not dma_only:
                    idxall = fpool.tile([P, NTILES], mybir.dt.uint32, tag="idxb")
                    ftl = []
                cfused = None
                fall4 = None
                if fuse_gather and not dma_only:
                    assert interleave4
                    fall4 = ftp.tile(
                        [P, NTILES * FEAT_DIM], mybir.dt.float32, tag="f4"
                    )
                    nc.sync.dma_start(
                        out=fall4[:],
                        in_=feats[:].rearrange("(q f) d -> q (f d)", f=NTILES),
                    )
                    cfused = []
                    for jj in range(NTILES):
                        ct = ftp.tile(
                            [P, FEAT_DIM],
                            mybir.dt.float32,
                            tag=f"ct{jj}",
                            name=f"ct{jj}",
                        )
                        nc.scalar.activation(
                            ct[:],
                            fall4[:, jj * FEAT_DIM : (jj + 1) * FEAT_DIM],
                            mybir.ActivationFunctionType.Copy,
                            scale=-1.0,
                        )
                        cfused.append(ct)
                for j in range(NTILES):
                    rows = slice(j * P, (j + 1) * P)

                    if dma_only:
                        # aligned-pitch tile, same as the hierg data path
                        ptile = pred_pool.tile([P, 6656], mybir.dt.float32)
                        _load_pred(ptile, rows, j)
                        nc.vector.reduce_max(
                            dacc[:, j : j + 1], ptile[:, 0:8],
                            axis=mybir.AxisListType.X,
                        )
                        continue

                    if pass1 == "hierg" and not dma_only:
                        if persist_pred:
                            ptile = ptiles[(_rep * NTILES + j) % pred_bufs]
                            _load_pred(ptile, rows, j)
                        else:
                            # padded to (6656/seg_w) groups x seg_w cols; pad
                            # filled on ACT to keep the vector engine free
                            ptile = pred_pool.tile([P, 6656], mybir.dt.float32)
                            _load_pred(ptile, rows, j)
                            nc.scalar.activation(
                                ptile[:, NUM_CLASSES:],
                                ones[:, 0:1].to_broadcast([P, 6656 - NUM_CLASSES]),
                                mybir.ActivationFunctionType.Copy,
                                scale=-1e30,
                            )
                    else:
                        ptile = pred_pool.tile([P, NUM_CLASSES], mybir.dt.float32)
                        nc.sync.dma_start(out=ptile[:], in_=preds[rows, :])

                    if dma_only:
                        # touch a sliver so the load isn't dead
                        nc.vector.reduce_max(
                            dacc[:, j : j + 1], ptile[:, 0:8],
                            axis=mybir.AxisListType.X,
                        )
                        continue

                    if pass1 == "hierg":
                        # groups of seg_w columns (padded with -1e30 so class
                        # index == g*seg_w + k, recoverable with bit ops)
                        SP = seg_w
                        GP = 6656 // SP
                        SHIFT = SP.bit_length() - 1
                        assert 1 << SHIFT == SP
                        gmax = small_pool.tile([P, GP], mybir.dt.float32)
                        nc.vector.reduce_max(
                            gmax[:],
                            ptile[:].rearrange("p (g k) -> p g k", k=SP),
                            axis=mybir.AxisListType.X,
                        )
                        # top-8 group maxes, sorted desc: slot 0 is the row
                        # max, and the tile doubles as max_index's in_max
                        rmax8 = small_pool.tile([P, 8], mybir.dt.float32)
                        nc.vector.max(rmax8[:], gmax[:])
                        g8 = small_pool.tile([P, 8], mybir.dt.uint16)
                        nc.vector.max_index(g8[:], rmax8[:], gmax[:])
                        # gather each partition's winning 128-wide group from
                        # SBUF on GpSimd. ap_gather broadcasts each of the 16
                        # partition indices to the whole 16-partition group, so
                        # partition p's own winning group lands at block p%16;
                        # the row max value only occurs in p's own group (any
                        # duplicate block is that same group, with the value at
                        # the same in-block position), so a value search over
                        # all 16 blocks yields the right in-group position.
                        blocks = small_pool.tile([P, 16, SP], mybir.dt.float32)
                        nc.gpsimd.ap_gather(
                            out_ap=blocks[:],
                            in_ap=ptile[:].rearrange("p (g k) -> p g k", k=SP),
                            idxs_ap=g8[:, 0:1].bitcast(mybir.dt.int16),
                            channels=P,
                            num_elems=GP,
                            d=SP,
                            num_idxs=16,
                        )
                        f8 = small_pool.tile([P, 8], mybir.dt.uint16)
                        nc.vector.max_index(
                            f8[:],
                            rmax8[:],
                            blocks[:].rearrange("p a b -> p (a b)"),
                        )
                        # class index = (g << SHIFT) + (found & (SP-1))
                        kmod = small_pool.tile([P, 1], mybir.dt.uint16)
                        nc.vector.tensor_scalar(
                            out=kmod[:],
                            in0=f8[:, 0:1],
                            scalar1=SP - 1,
                            scalar2=None,
                            op0=mybir.AluOpType.bitwise_and,
                        )
                        gshl = small_pool.tile([P, 1], mybir.dt.uint16)
                        nc.vector.tensor_scalar(
                            out=gshl[:],
                            in0=g8[:, 0:1],
                            scalar1=SHIFT,
                            scalar2=None,
                            op0=mybir.AluOpType.logical_shift_left,
                        )
                        idx8 = small_pool.tile([P, 1], mybir.dt.uint32)
                        nc.vector.tensor_tensor(
                            out=idx8[:],
                            in0=gshl[:],
                            in1=kmod[:],
                            op=mybir.AluOpType.add,
                        )
                    elif pass1 == "hier":
                        gmax = small_pool.tile([P, G], mybir.dt.float32)
                        nc.vector.reduce_max(
                            gmax[:],
                            ptile[:].rearrange("p (g k) -> p g k", k=SEG),
                            axis=mybir.AxisListType.X,
                        )
                        rmax = small_pool.tile([P, 1], mybir.dt.float32)
                        nc.vector.reduce_max(
                            rmax[:], gmax[:], axis=mybir.AxisListType.X
                        )
                        rmax8 = small_pool.tile([P, 8], mybir.dt.float32)
                        nc.scalar.activation(
                            rmax8[:],
                            rmax[:].to_broadcast([P, 8]),
                            mybir.ActivationFunctionType.Copy,
                        )
                        g8 = small_pool.tile([P, 8], mybir.dt.uint32)
                        nc.vector.max_index(g8[:], rmax8[:], gmax[:])
                        g8f = small_pool.tile([P, 1], mybir.dt.float32)
                        nc.vector.tensor_copy(g8f[:], g8[:, 0:1])
                        # offset into preds_flat: row*G + g
                        offsf = small_pool.tile([P, 1], mybir.dt.float32)
                        nc.scalar.activation(
                            offsf[:],
                            g8f[:],
                            mybir.ActivationFunctionType.Identity,
                            bias=rowbase_f[:, j : j + 1],
                        )
                        offsi = small_pool.tile([P, 1], mybir.dt.int32)
                        nc.vector.tensor_copy(offsi[:], offsf[:])
                        seg = small_pool.tile([P, SEG], mybir.dt.float32)
                        if fake_gather:
                            # benchmark probe: fixed-window read instead of a
                            # per-row indirect gather (wrong results)
                            nc.vector.tensor_copy(seg[:], ptile[:, 0:SEG])
                        else:
                            nc.gpsimd.indirect_dma_start(
                                out=seg[:],
                                out_offset=None,
                                in_=preds_flat,
                                in_offset=bass.IndirectOffsetOnAxis(
                                    ap=offsi[:, 0:1], axis=0
                                ),
                            )
                        k8 = small_pool.tile([P, 8], mybir.dt.uint32)
                        nc.vector.max_index(k8[:], rmax8[:], seg[:])
                        k8f = small_pool.tile([P, 1], mybir.dt.float32)
                        nc.vector.tensor_copy(k8f[:], k8[:, 0:1])
                        # class index: g*SEG + k
                        idxf = small_pool.tile([P, 1], mybir.dt.float32)
                        nc.scalar.activation(
                            idxf[:],
                            g8f[:],
                            mybir.ActivationFunctionType.Identity,
                            scale=float(SEG),
                            bias=k8f[:],
                        )
                        idx8 = small_pool.tile([P, 1], mybir.dt.uint32)
                        nc.vector.tensor_copy(idx8[:], idxf[:])
                    else:
                        max8 = small_pool.tile([P, 8], mybir.dt.float32)
                        idx8 = small_pool.tile([P, 8], mybir.dt.uint32)
                        nc.vector.max(max8[:], ptile[:])
                        nc.vector.max_index(idx8[:], max8[:], ptile[:])

                    if stage <= 1:
                        # probe: argmax only; fold idx into dacc so it's live
                        nc.vector.tensor_copy(dacc[:, j : j + 1], idx8[:, 0:1])
                        continue

                    if batch_cgather:
                        nc.vector.tensor_copy(idxall[:, j : j + 1], idx8[:, 0:1])
                        continue

                    if batch_cgather2:
                        nc.vector.tensor_copy(idxall[:, j : j + 1], idx8[:, 0:1])
                        ftile_t = ftp.tile(
                            [P, FEAT_DIM], mybir.dt.float32, tag=f"ft{j}"
                        )
                        feng = (
                            {"gp": nc.gpsimd, "act": nc.scalar}[feat_eng]
                            if feat_eng
                            else small_eng
                        )
                        feng.dma_start(out=ftile_t[:], in_=feats[rows, :])
                        ftl.append(ftile_t)
                        continue

                    if pass1 == "hierg" and batch_feat:
                        # all 512 feature rows in one strided DMA per rep
                        if j == 0:
                            fall = small_pool.tile(
                                [P, NTILES, FEAT_DIM], mybir.dt.float32, tag="fall"
                            )
                            nc.sync.dma_start(
                                out=fall[:],
                                in_=feats[:].rearrange("(j p) d -> p j d", p=P),
                            )
                        ftile = fall[:, j, :]
                    elif feat_first:
                        ftile = fall_t[:, j, :]
                    elif interleave4:
                        if j == 0 and not fuse_gather:
                            fall4 = ftp.tile(
                                [P, NTILES * FEAT_DIM], mybir.dt.float32, tag="f4"
                            )
                            nc.sync.dma_start(
                                out=fall4[:],
                                in_=feats[:].rearrange(
                                    "(q f) d -> q (f d)", f=NTILES
                                ),
                            )
                        ftile = fall4[:, j * FEAT_DIM : (j + 1) * FEAT_DIM]
                    elif stage >= 3:
                        fp = ftp if fbufs > 0 else small_pool
                        ftile_t = fp.tile([P, FEAT_DIM], mybir.dt.float32)
                        feng = (
                            {"gp": nc.gpsimd, "act": nc.scalar}[feat_eng]
                            if feat_eng
                            else small_eng
                        )
                        feng.dma_start(out=ftile_t[:], in_=feats[rows, :])
                        ftile = ftile_t[:]

                    if fuse_gather:
                        nc.gpsimd.indirect_dma_start(
                            out=cfused[j][:],
                            out_offset=None,
                            in_=cents[:],
                            in_offset=bass.IndirectOffsetOnAxis(
                                ap=idx8[:, 0:1], axis=0
                            ),
                            compute_op=mybir.AluOpType.add,
                        )
                        sq = small_pool.tile([P, FEAT_DIM], mybir.dt.float32)
                        nc.scalar.activation(
                            sq[:],
                            cfused[j][:],
                            mybir.ActivationFunctionType.Square,
                            accum_out=dacc[:, j : j + 1],
                        )
                        continue

                    ctile = small_pool.tile([P, FEAT_DIM], mybir.dt.float32)
                    if fake_gather:
                        nc.sync.dma_start(out=ctile[:], in_=cents[0:P, :])
                    else:
                        nc.gpsimd.indirect_dma_start(
                            out=ctile[:],
                            out_offset=None,
                            in_=cents[:],
                            in_offset=bass.IndirectOffsetOnAxis(
                                ap=idx8[:, 0:1], axis=0
                            ),
                        )

                    if stage == 2:
                        # probe: skip feats load/diff, square the gathered rows
                        sq = small_pool.tile([P, FEAT_DIM], mybir.dt.float32)
                        nc.scalar.activation(
                            sq[:],
                            ctile[:],
                            mybir.ActivationFunctionType.Square,
                            accum_out=dacc[:, j : j + 1],
                        )
                        continue

                    diff = small_pool.tile([P, FEAT_DIM], mybir.dt.float32)
                    nc.vector.tensor_tensor(
                        out=diff[:],
                        in0=ftile,
                        in1=ctile[:],
                        op=mybir.AluOpType.subtract,
                    )
                    sq = small_pool.tile([P, FEAT_DIM], mybir.dt.float32)
                    nc.scalar.activation(
                        sq[:],
                        diff[:],
                        mybir.ActivationFunctionType.Square,
                        accum_out=dacc[:, j : j + 1],
                    )

                if batch_cgather2 and not dma_only:
                    idxall2 = fpool.tile([P, NTILES], mybir.dt.uint32, tag="idxb2")
                    nc.vector.tensor_copy(idxall2[:], idxall[:])
                    callb = fpool.tile(
                        [P, NTILES, FEAT_DIM], mybir.dt.float32, tag="callb2"
                    )
                    nc.gpsimd.indirect_dma_start(
                        out=callb[:],
                        out_offset=None,
                        in_=cents[:],
                        in_offset=bass.IndirectOffsetOnAxis(
                            ap=idxall2[:, 0:NTILES], axis=0
                        ),
                    )
                    for j in range(NTILES):
                        diff = small_pool.tile([P, FEAT_DIM], mybir.dt.float32)
                        nc.vector.tensor_tensor(
                            out=diff[:],
                            in0=ftl[j][:],
                            in1=callb[:, j, :],
                            op=mybir.AluOpType.subtract,
                        )
                        sq = small_pool.tile([P, FEAT_DIM], mybir.dt.float32)
                        nc.scalar.activation(
                            sq[:],
                            diff[:],
                            mybir.ActivationFunctionType.Square,
                            accum_out=dacc[:, j : j + 1],
                        )
                if batch_cgather and not dma_only:
                    call = fpool.tile(
                        [P, NTILES, FEAT_DIM], mybir.dt.float32, tag="call2"
                    )
                    nc.gpsimd.indirect_dma_start(
                        out=call[:],
                        out_offset=None,
                        in_=cents[:],
                        in_offset=bass.IndirectOffsetOnAxis(
                            ap=idxall[:, 0:NTILES], axis=0
                        ),
                    )
                    for j in range(NTILES):
                        diff = small_pool.tile([P, FEAT_DIM], mybir.dt.float32)
                        nc.vector.tensor_tensor(
                            out=diff[:],
                            in0=fall_t[:, j, :],
                            in1=call[:, j, :],
                            op=mybir.AluOpType.subtract,
                        )
                        sq = small_pool.tile([P, FEAT_DIM], mybir.dt.float32)
                        nc.scalar.activation(
                            sq[:],
                            diff[:],
                            mybir.ActivationFunctionType.Square,
                            accum_out=dacc[:, j : j + 1],
                        )
                oeng = {"sp": nc.sync, "act": nc.scalar, "gp": nc.gpsimd}[out_eng]
                _final_reduce(
                    nc, persist_pool, psum_pool, dacc, ones, out, eng=oeng
                )

    nc.compile()
    return nc


def _build_nc_v2(
    reps=1,
    npair_bufs=3,
    out_eng="gp",
    cg_batch=False,
):
    """Pair-load layout: tile j covers samples {4q+j} (partition q). Tiles
    {0,1} and {2,3} are adjacent DRAM rows per partition, so each rep does
    just 2 big predicts DMAs ([P, 2, 6625], ~53KB/partition) + 1 contiguous
    feats DMA. Pad columns are persistent (filled once). Argmax per sub-tile
    is the hierg chain (group reduce_max + top8 + ap_gather + bit ops)."""
    nc = bacc.Bacc("TRN2", target_bir_lowering=False)
    feats = nc.dram_tensor(
        "features", [NS, FEAT_DIM], mybir.dt.float32, kind="ExternalInput"
    )
    preds = nc.dram_tensor(
        "predicts", [NS, NUM_CLASSES], mybir.dt.float32, kind="ExternalInput"
    )
    cents = nc.dram_tensor(
        "centers", [NUM_CLASSES, FEAT_DIM], mybir.dt.float32, kind="ExternalInput"
    )
    out = nc.dram_tensor("out", [1, 1], mybir.dt.float32, kind="ExternalOutput")

    SP = 128
    GP = 6656 // SP  # 52 groups
    SHIFT = SP.bit_length() - 1

    with TileContext(nc) as tc:
        with (
            tc.tile_pool(name="small", bufs=3) as small_pool,
            tc.tile_pool(name="ftp", bufs=2) as ftp,
            tc.tile_pool(name="persist", bufs=1) as persist_pool,
            tc.tile_pool(name="psum", bufs=1, space="PSUM") as psum_pool,
        ):
            ones = persist_pool.tile([P, 1], mybir.dt.float32)
            nc.vector.memset(ones[:], 1.0)

            oeng = {"sp": nc.sync, "act": nc.scalar, "gp": nc.gpsimd}[out_eng]

            pts = []
            for i in range(npair_bufs):
                ppt = persist_pool.tile(
                    [P, 2, 6656], mybir.dt.float32, tag=f"pp{i}", name=f"pp{i}"
                )
                for t in range(2):
                    nc.scalar.activation(
                        ppt[:, t, NUM_CLASSES:],
                        ones[:, 0:1].to_broadcast([P, 6656 - NUM_CLASSES]),
                        mybir.ActivationFunctionType.Copy,
                        scale=-1e30,
                    )
                pts.append(ppt)

            preds4 = preds[:].rearrange("(q f) c -> q f c", f=NTILES)

            for _rep in range(reps):
                dacc = persist_pool.tile([P, NTILES], mybir.dt.float32, tag="dacc")
                fall4 = ftp.tile([P, NTILES * FEAT_DIM], mybir.dt.float32)
                nc.sync.dma_start(
                    out=fall4[:],
                    in_=feats[:].rearrange("(q f) d -> q (f d)", f=NTILES),
                )
                idxall = None
                if cg_batch:
                    idxall = ftp.tile([P, NTILES], mybir.dt.uint32, tag="idxa")
                for pair in range(2):
                    ppt = pts[(_rep * 2 + pair) % npair_bufs]
                    nc.sync.dma_start(
                        out=ppt[:, :, 0:NUM_CLASSES],
                        in_=preds4[:, 2 * pair : 2 * pair + 2, :],
                    )
                    for t in range(2):
                        j = 2 * pair + t
                        ptv = ppt[:, t, :]
                        ptv3 = ppt[:, t, :].rearrange("p (g k) -> p g k", k=SP)
                        gmax = small_pool.tile([P, GP], mybir.dt.float32)
                        nc.vector.reduce_max(
                            gmax[:], ptv3, axis=mybir.AxisListType.X
                        )
                        rmax8 = small_pool.tile([P, 8], mybir.dt.float32)
                        nc.vector.max(rmax8[:], gmax[:])
                        g8 = small_pool.tile([P, 8], mybir.dt.uint16)
                        nc.vector.max_index(g8[:], rmax8[:], gmax[:])
                        blocks = small_pool.tile([P, 16, SP], mybir.dt.float32)
                        nc.gpsimd.ap_gather(
                            out_ap=blocks[:],
                            in_ap=ptv3,
                            idxs_ap=g8[:, 0:1].bitcast(mybir.dt.int16),
                            channels=P,
                            num_elems=GP,
                            d=SP,
                            num_idxs=16,
                        )
                        f8 = small_pool.tile([P, 8], mybir.dt.uint16)
                        nc.vector.max_index(
                            f8[:],
                            rmax8[:],
                            blocks[:].rearrange("p a b -> p (a b)"),
                        )
                        kmod = small_pool.tile([P, 1], mybir.dt.uint16)
                        nc.vector.tensor_scalar(
                            out=kmod[:],
                            in0=f8[:, 0:1],
                            scalar1=SP - 1,
                            scalar2=None,
                            op0=mybir.AluOpType.bitwise_and,
                        )
                        gshl = small_pool.tile([P, 1], mybir.dt.uint16)
                        nc.vector.tensor_scalar(
                            out=gshl[:],
                            in0=g8[:, 0:1],
                            scalar1=SHIFT,
                            scalar2=None,
                            op0=mybir.AluOpType.logical_shift_left,
                        )
                        idx8 = small_pool.tile([P, 1], mybir.dt.uint32)
                        nc.vector.tensor_tensor(
                            out=idx8[:],
                            in0=gshl[:],
                            in1=kmod[:],
                            op=mybir.AluOpType.add,
                        )
                        if cg_batch:
                            nc.vector.tensor_copy(
                                idxall[:, j : j + 1], idx8[:, 0:1]
                            )
                            continue
                        ctile = small_pool.tile([P, FEAT_DIM], mybir.dt.float32)
                        nc.gpsimd.indirect_dma_start(
                            out=ctile[:],
                            out_offset=None,
                            in_=cents[:],
                            in_offset=bass.IndirectOffsetOnAxis(
                                ap=idx8[:, 0:1], axis=0
                            ),
                        )
                        diff = small_pool.tile([P, FEAT_DIM], mybir.dt.float32)
                        nc.vector.tensor_tensor(
                            out=diff[:],
                            in0=fall4[:, j * FEAT_DIM : (j + 1) * FEAT_DIM],
                            in1=ctile[:],
                            op=mybir.AluOpType.subtract,
                        )
                        sq = small_pool.tile([P, FEAT_DIM], mybir.dt.float32)
                        nc.scalar.activation(
                            sq[:],
                            diff[:],
                            mybir.ActivationFunctionType.Square,
                            accum_out=dacc[:, j : j + 1],
                        )
                if cg_batch:
                    call = ftp.tile(
                        [P, NTILES, FEAT_DIM], mybir.dt.float32, tag="callb"
                    )
                    nc.gpsimd.indirect_dma_start(
                        out=call[:],
                        out_offset=None,
                        in_=cents[:],
                        in_offset=bass.IndirectOffsetOnAxis(
                            ap=idxall[:, 0:NTILES], axis=0
                        ),
                    )
                    for j in range(NTILES):
                        diff = small_pool.tile([P, FEAT_DIM], mybir.dt.float32)
                        nc.vector.tensor_tensor(
                            out=diff[:],
                            in0=fall4[:, j * FEAT_DIM : (j + 1) * FEAT_DIM],
                            in1=call[:, j, :],
                            op=mybir.AluOpType.subtract,
                        )
                        sq = small_pool.tile([P, FEAT_DIM], mybir.dt.float32)
                        nc.scalar.activation(
                            sq[:],
                            diff[:],
                            mybir.ActivationFunctionType.Square,
                            accum_out=dacc[:, j : j + 1],
                        )
                _final_reduce(nc, persist_pool, psum_pool, dacc, ones, out, eng=oeng)

    nc.compile()
    return nc


def _final_reduce(nc, persist_pool, psum_pool, dacc, ones, out, eng=None):
    dclamp = persist_pool.tile([P, NTILES], mybir.dt.float32, tag="dclamp")
    nc.vector.tensor_scalar(
        out=dclamp[:],
        in0=dacc[:],
        scalar1=CLAMP_MIN,
        scalar2=CLAMP_MAX,
        op0=mybir.AluOpType.max,
        op1=mybir.AluOpType.min,
    )
    dsum = persist_pool.tile([P, 1], mybir.dt.float32, tag="dsum")
    nc.vector.reduce_sum(dsum[:], dclamp[:], axis=mybir.AxisListType.X)

    res_psum = psum_pool.tile([1, 1], mybir.dt.float32)
    nc.tensor.matmul(res_psum[:], lhsT=dsum[:], rhs=ones[:], start=True, stop=True)
    res_sb = persist_pool.tile([1, 1], mybir.dt.float32, tag="res_sb")
    nc.vector.tensor_copy(res_sb[:], res_psum[:])
    (eng or nc.sync).dma_start(out=out[:], in_=res_sb[:])


# Best measured configuration (interleaved A/B, 100 trials): single
# contiguous feats DMA (interleave4 sample mapping), out store on the SWDGE
# queue so it never stalls the SP predicts stream, persistent predicts
# buffers with one-time -1e30 pad fill.
BEST_KWARGS = dict(
    interleave4=True,
    fbufs=2,
    out_eng="gp",
    persist_pred=True,
    pred_bufs=4,
    # 64-wide argmax groups: halves the in-group value-search width (16x64
    # vs 16x128 elements) on the DVE critical chain; measured -2.9us vs 128.
    seg_w=64,
)


def _get_nc():
    if "nc" not in _NC_CACHE:
        _NC_CACHE["nc"] = _build_nc(**BEST_KWARGS)
    return _NC_CACHE["nc"]


def kernel(features, predicts, centers):
    features = np.ascontiguousarray(np.asarray(features, dtype=np.float32))
    predicts = np.ascontiguousarray(np.asarray(predicts, dtype=np.float32))
    centers = np.ascontiguousarray(np.asarray(centers, dtype=np.float32))

    feats = features.reshape(N_TOTAL, FEAT_DIM)
    preds = predicts.reshape(N_TOTAL, NUM_CLASSES)

    in_maps = []
    for c in range(N_CORES):
        rows = slice(c * NS, (c + 1) * NS)
        in_maps.append(
            {
                "features": np.ascontiguousarray(feats[rows]),
                "predicts": np.ascontiguousarray(preds[rows]),
                "centers": centers,
            }
        )

    nc = _get_nc()
    res = run_bass_kernel_spmd(nc, in_maps, list(range(N_CORES)))
    partial = np.array(
        [res.results[i]["out"][0, 0] for i in range(N_CORES)], dtype=np.float64
    )
    loss = partial.sum() / N_TOTAL + (NUM_CLASSES - 1) * CLAMP_MIN
    return np.float64(loss)

